# revision 3
# baseline (speedup 1.0000x reference)
"""Trainium2 Bass kernel for nn_Program_4578435138231.

Sharding: pure data parallelism over the batch dim (262144 rows) across 8
NeuronCores; the tiny classifier weights are replicated (pre-broadcast to
128 partitions host-side).

Per core (R=32768 rows): rows are processed in CHUNKS of 128*FC rows laid
out row-major in an SBUF "out_tile" (178 fp32 per row) so the final DMA to
DRAM is one contiguous run per partition. The 15-step scan runs batch-major:
partition = batch lane, free dim = FC rows per partition; state columns are
[128, FC] tiles / strided views into out_tile.
"""

from contextlib import ExitStack

import numpy as np

import bass_rust
import concourse.bass as bass
import concourse.tile as tile
from concourse import mybir
from concourse.bass_utils import run_bass_kernel_spmd
from concourse.vector_clock import ScopedClock

# ---------------------------------------------------------------- constants
N_CORES = 8
B = 262144
R = B // N_CORES          # rows per core
FC = 128                  # rows per partition per chunk
CHUNK_ROWS = 128 * FC
N_CHUNKS = R // CHUNK_ROWS
STEPS = 15
SPEED = 5.0
NCOL = 178                # 18 state + 160 traj
F32 = mybir.dt.float32
OP = mybir.AluOpType
AF = mybir.ActivationFunctionType

# wconst layout (per-partition replicated scalars)
O_C1W0, O_C1W1, O_C1B = 0, 1, 2
O_C2W0, O_C2W1, O_C2B = 3, 4, 5
O_L1W = 6                 # [4,32] k*32+o
O_L1B = O_L1W + 128       # 134..165
O_L2WT = O_L1B + 32       # 166..325  l2w transposed: c*32+o
O_L2B = O_L2WT + 160      # 326..330
NW = O_L2B + 5            # 331


# --------------------------------------------------- tail-drain split patch
def _split_drain_and_barrier(self, tick_clock, wait_clock):
    nc = self.nc
    drain_inst = nc.sync.drain()
    wait_clock.add_sem_waits(
        drain_inst.ins, ScopedClock({None: tick_clock.global_clock})
    )
    si = drain_inst.ins.sync_info
    waits = list(si.on_wait) if si is not None else []
    if len(waits) > 1:
        updates = list(si.on_update) if si is not None else []
        drain_inst.ins.sync_info = bass_rust.SyncInfo(
            on_wait=[waits[0]], on_update=updates
        )
        for w in waits[1:]:
            extra = nc.sync.drain()
            extra.ins.sync_info = bass_rust.SyncInfo(on_wait=[w], on_update=[])
    nc.all_engine_barrier()
    assert self.sems is not None
    popped = nc._tile_sem_poison_stack.pop()
    assert popped is self._sem_poison
    nc.clear_and_free_semaphores(list(self.sems.allocated().values()))
    nc.all_engine_barrier()


tile.TileContext._drain_and_barrier = _split_drain_and_barrier


def _split_multiwait_instructions(nc):
    """This walrus build rejects >1 sync-wait per instruction; hoist extra
    waits onto same-engine NOPs inserted immediately before the owner."""
    for fn in nc.m.functions:
        for bb in fn.blocks:
            insts = bb.instructions
            i = 0
            while i < len(insts):
                inst = insts[i]
                si = getattr(inst, "sync_info", None)
                waits = list(si.on_wait) if si is not None else []
                if len(waits) > 1:
                    updates = list(si.on_update)
                    for w in waits[:-1]:
                        nop = mybir.InstNoOp(
                            name=nc.get_next_instruction_name(), ins=[], outs=[]
                        )
                        nop.engine = inst.engine
                        nop.sync_info = bass_rust.SyncInfo(
                            on_wait=[w], on_update=[]
                        )
                        nc.register_instruction(nop, overwrite=True)
                        insts.insert(i, nop)
                        i += 1
                    inst.sync_info = bass_rust.SyncInfo(
                        on_wait=[waits[-1]], on_update=updates
                    )
                i += 1


# ------------------------------------------------------------- bass program
def _emit_chunk(nc, pools, xin, wc, out_t):
    """Emit the 15-step program for one chunk.

    xin: [128, FC*18] input rows (row-major, stride 18)
    wc:  [128, NW] replicated weight scalars
    out_t: [128, FC*178] output rows (row-major, stride 178)
    """
    v = nc.vector
    s = nc.scalar
    Fc = FC

    def ws(i):  # [128,1] scalar AP
        return wc[:, i : i + 1]

    xin3 = xin.rearrange("p (j d) -> p j d", d=18)
    out3 = out_t.rearrange("p (j d) -> p j d", d=NCOL)

    def xcol(c):            # [128, FC] stride-18 view of x column c
        return xin3[:, :, c]

    def ocol(c):            # [128, FC] stride-178 view of out column c
        return out3[:, :, c]

    work = pools["work"]
    feat_a = work.tile([128, 6 * Fc], F32, tag="feat_a")
    feat_b = work.tile([128, 6 * Fc], F32, tag="feat_b")
    h1 = work.tile([128, 5 * Fc], F32, tag="h1")
    h2 = work.tile([128, 4 * Fc], F32, tag="h2")
    h3 = work.tile([128, 32 * Fc], F32, tag="h3")
    prod = work.tile([128, 32 * Fc], F32, tag="prod")
    pbuf = work.tile([128, 4 * Fc], F32, tag="pbuf")
    mk = work.tile([128, 6 * Fc], F32, tag="mk")       # masks A..F
    df = work.tile([128, 6 * Fc], F32, tag="df")       # diffs a..f
    t0 = work.tile([128, Fc], F32, tag="t0")
    t1 = work.tile([128, Fc], F32, tag="t1")
    t2 = work.tile([128, Fc], F32, tag="t2")
    t3 = work.tile([128, Fc], F32, tag="t3")
    s4c = work.tile([128, Fc], F32, tag="s4c")         # static s4 col

    def fslot(buf, i):
        return buf[:, i * Fc : (i + 1) * Fc]

    # ---- init: traj0 + static cols ----
    # s0 final = x0 + 15
    v.tensor_scalar_add(ocol(0), xcol(0), float(STEPS))
    # s4 (static)
    v.tensor_copy(s4c[:], xcol(4))
    v.tensor_copy(ocol(4), xcol(4))
    # dist0 = (x1-x3)^2 + (x2-x4)^2 -> traj0 col 0
    v.tensor_tensor(t0[:], xcol(1), xcol(3), OP.subtract)
    v.tensor_tensor(t0[:], t0[:], t0[:], OP.mult)
    v.tensor_tensor(t1[:], xcol(2), xcol(4), OP.subtract)
    v.tensor_tensor(t1[:], t1[:], t1[:], OP.mult)
    v.tensor_tensor(ocol(18 + 0), t0[:], t1[:], OP.add)
    # traj0 cols 1..9 = x[1,2,3,4,5,6,7,8,17]
    for k, c in enumerate([1, 2, 3, 4, 5, 6, 7, 8, 17]):
        s.copy(ocol(18 + 1 + k), xcol(c))
    # initial feat buffer (feat_a): [s1,s2,s3,s4,s9,s17]
    for k, c in enumerate([1, 2, 3, 4, 9, 17]):
        v.tensor_copy(fslot(feat_a, k), xcol(c))

    h33 = h3.rearrange("p (j o) -> p j o", o=32)
    prod3 = prod.rearrange("p (j o) -> p j o", o=32)

    cur, nxt = feat_a, feat_b
    for t in range(1, STEPS + 1):
        last = t == STEPS
        tb = 18 + 10 * t  # traj block base col

        # ---- classifier ----
        # conv1: h1 = relu(c1w0*f[j] + c1w1*f[j+1] + c1b), vectorized over 5
        A = cur[:, 0 : 5 * Fc]
        Bv = cur[:, Fc : 6 * Fc]
        v.tensor_scalar_mul(h1[:], Bv, ws(O_C1W1))
        v.scalar_tensor_tensor(h1[:], A, ws(O_C1W0), h1[:], OP.mult, OP.add)
        s.activation(h1[:], h1[:], AF.Relu, bias=ws(O_C1B))
        # conv2: 4 outs
        v.tensor_scalar_mul(h2[:], h1[:, Fc : 5 * Fc], ws(O_C2W1))
        v.scalar_tensor_tensor(
            h2[:], h1[:, 0 : 4 * Fc], ws(O_C2W0), h2[:], OP.mult, OP.add
        )
        s.activation(h2[:], h2[:], AF.Relu, bias=ws(O_C2B))
        # dense1: z[j,o] = sum_k h2_k[j] * l1w[k,o] + l1b[o]; relu
        for k in range(4):
            h2k = fslot(h2, k).unsqueeze(-1).broadcast_to((128, Fc, 32))
            wrow = (
                wc[:, O_L1W + 32 * k : O_L1W + 32 * (k + 1)]
                .unsqueeze(1)
                .broadcast_to((128, Fc, 32))
            )
            if k == 0:
                v.tensor_tensor(h33[:], h2k, wrow, OP.mult)
            else:
                v.tensor_tensor(prod3[:], h2k, wrow, OP.mult)
                v.tensor_tensor(h33[:], h33[:], prod3[:], OP.add)
        b3 = wc[:, O_L1B : O_L1B + 32].unsqueeze(1).broadcast_to((128, Fc, 32))
        v.tensor_tensor(h33[:], h33[:], b3, OP.add)
        v.tensor_scalar_max(h3[:], h3[:], 0.0)
        # dense2 + sigmoid: p_c = sigmoid(sum_o h3[j,o]*l2w[o,c] + l2b[c])
        for c in range(5):
            w4c = (
                wc[:, O_L2WT + 32 * c : O_L2WT + 32 * (c + 1)]
                .unsqueeze(1)
                .broadcast_to((128, Fc, 32))
            )
            v.tensor_tensor(prod3[:], h33[:], w4c, OP.mult)
            v.tensor_reduce(t0[:], prod3[:], mybir.AxisListType.X, OP.add)
            dst = fslot(pbuf, c) if c < 4 else fslot(nxt, 5)
            s.activation(dst, t0[:], AF.Sigmoid, bias=ws(O_L2B + c))
        # traj cols 5..9 = p0..p4
        for c in range(4):
            s.copy(ocol(tb + 5 + c), fslot(pbuf, c))
        s.copy(ocol(tb + 9), fslot(nxt, 5))
        if last:
            for c in range(4):
                s.copy(ocol(5 + c), fslot(pbuf, c))
            s.copy(ocol(17), fslot(nxt, 5))

        # ---- comparisons a..f -> diffs, masks ----
        pairs = [(1, 0), (2, 0), (3, 0), (2, 1), (3, 1), (3, 2)]
        for i, (hi, lo) in enumerate(pairs):
            ddst = ocol(11 + i) if last else fslot(df, i)
            v.tensor_tensor(ddst, fslot(pbuf, hi), fslot(pbuf, lo), OP.subtract)
            v.tensor_scalar(fslot(mk, i), ddst, 0.0, None, OP.is_gt)
        MA, MB, MC, MD, ME, MF = (fslot(mk, i) for i in range(6))

        # ---- dx', st via mask algebra ----
        # dx_b = MC + MB*(MF-MC); dx_e = 2*ME-1; dx_d = dx_e + MD*(MF-dx_e)
        # dx' = dx_b + MA*(dx_d - dx_b); s1' = s1 + 5*dx'
        v.tensor_tensor(t0[:], MF, MC, OP.subtract)
        v.tensor_tensor(t0[:], MB, t0[:], OP.mult)
        v.tensor_tensor(t0[:], t0[:], MC, OP.add)          # t0 = dx_b
        v.tensor_scalar(t1[:], ME, 2.0, -1.0, OP.mult, OP.add)  # t1 = dx_e
        v.tensor_tensor(t2[:], MF, t1[:], OP.subtract)
        v.tensor_tensor(t2[:], MD, t2[:], OP.mult)
        v.tensor_tensor(t2[:], t2[:], t1[:], OP.add)       # t2 = dx_d
        v.tensor_tensor(t2[:], t2[:], t0[:], OP.subtract)
        v.tensor_tensor(t2[:], MA, t2[:], OP.mult)
        v.tensor_tensor(t2[:], t2[:], t0[:], OP.add)       # t2 = dx'
        v.scalar_tensor_tensor(
            fslot(nxt, 0), t2[:], SPEED, fslot(cur, 0), OP.mult, OP.add
        )  # s1'
        # st_b = 3MC + MB*(stf-3MC); stf = MF+2; ste = 2ME+1
        # st_d = ste + MD*(stf-ste); st = st_b + MA*(st_d-st_b)
        v.tensor_scalar_mul(t0[:], MC, 3.0)                # t0 = st_c
        v.tensor_scalar_add(t1[:], MF, 2.0)                # t1 = st_f
        v.tensor_tensor(t2[:], t1[:], t0[:], OP.subtract)
        v.tensor_tensor(t2[:], MB, t2[:], OP.mult)
        v.tensor_tensor(t0[:], t0[:], t2[:], OP.add)       # t0 = st_b
        v.tensor_scalar(t2[:], ME, 2.0, 1.0, OP.mult, OP.add)   # t2 = st_e
        v.tensor_tensor(t3[:], t1[:], t2[:], OP.subtract)
        v.tensor_tensor(t3[:], MD, t3[:], OP.mult)
        v.tensor_tensor(t2[:], t2[:], t3[:], OP.add)       # t2 = st_d
        v.tensor_tensor(t2[:], t2[:], t0[:], OP.subtract)
        v.tensor_tensor(t2[:], MA, t2[:], OP.mult)
        v.tensor_tensor(fslot(nxt, 4), t0[:], t2[:], OP.add)    # s9' = st

        # ---- deterministic updates ----
        v.tensor_scalar_add(fslot(nxt, 1), fslot(cur, 1), SPEED)  # s2'
        v.tensor_scalar_add(fslot(nxt, 2), fslot(cur, 2), SPEED)  # s3'
        # s4 passthrough into nxt
        v.tensor_copy(fslot(nxt, 3), s4c[:])
        # traj cols 1..4
        s.copy(ocol(tb + 1), fslot(nxt, 0))
        s.copy(ocol(tb + 2), fslot(nxt, 1))
        s.copy(ocol(tb + 3), fslot(nxt, 2))
        s.copy(ocol(tb + 4), s4c[:])
        # dist' = (s1'-s3')^2 + (s2'-s4)^2 -> traj col 0
        v.tensor_tensor(t0[:], fslot(nxt, 0), fslot(nxt, 2), OP.subtract)
        v.tensor_tensor(t0[:], t0[:], t0[:], OP.mult)
        v.tensor_tensor(t1[:], fslot(nxt, 1), s4c[:], OP.subtract)
        v.tensor_tensor(t1[:], t1[:], t1[:], OP.mult)
        v.tensor_tensor(ocol(tb + 0), t0[:], t1[:], OP.add)
        if last:
            # final state cols 1,2,3,9,10
            s.copy(ocol(1), fslot(nxt, 0))
            s.copy(ocol(2), fslot(nxt, 1))
            s.copy(ocol(3), fslot(nxt, 2))
            s.copy(ocol(9), fslot(nxt, 4))
            v.tensor_tensor(ocol(10), t0[:], t1[:], OP.add)

        cur, nxt = nxt, cur


def build_nc():
    nc = bass.Bass()
    x = nc.declare_dram_parameter("x", [R, 18], F32, isOutput=False)
    wc_d = nc.declare_dram_parameter("wconst", [128, NW], F32, isOutput=False)
    out = nc.declare_dram_parameter("out", [R, NCOL], F32, isOutput=True)

    xr = x.rearrange("(c p j) d -> c p (j d)", c=N_CHUNKS, p=128)
    outr = out.rearrange("(c p j) d -> c p (j d)", c=N_CHUNKS, p=128)

    with tile.TileContext(nc) as tc:
        with ExitStack() as ctx:
            pools = {
                "io": ctx.enter_context(tc.tile_pool(name="io", bufs=2)),
                "w": ctx.enter_context(tc.tile_pool(name="w", bufs=1)),
                "work": ctx.enter_context(tc.tile_pool(name="work", bufs=1)),
                "out": ctx.enter_context(tc.tile_pool(name="out", bufs=1)),
            }
            wc = pools["w"].tile([128, NW], F32)
            nc.sync.dma_start(wc[:], wc_d[:])
            for c in range(N_CHUNKS):
                xin = pools["io"].tile([128, FC * 18], F32, tag="xin")
                nc.sync.dma_start(xin[:], xr[c])
                out_t = pools["out"].tile([128, FC * NCOL], F32, tag="out_t")
                _emit_chunk(nc, pools, xin, wc[:], out_t[:])
                nc.sync.dma_start(outr[c], out_t[:])
    _split_multiwait_instructions(nc)
    return nc


_NC_CACHE = None


def _get_nc():
    global _NC_CACHE
    if _NC_CACHE is None:
        _NC_CACHE = build_nc()
    return _NC_CACHE


def _make_wconst(c1w, c1b, c2w, c2b, l1w, l1b, l2w, l2b):
    row = np.zeros(NW, np.float32)
    row[O_C1W0], row[O_C1W1], row[O_C1B] = c1w[0], c1w[1], c1b[0]
    row[O_C2W0], row[O_C2W1], row[O_C2B] = c2w[0], c2w[1], c2b[0]
    row[O_L1W : O_L1W + 128] = np.asarray(l1w, np.float32).reshape(-1)
    row[O_L1B : O_L1B + 32] = l1b
    row[O_L2WT : O_L2WT + 160] = np.asarray(l2w, np.float32).T.reshape(-1)
    row[O_L2B : O_L2B + 5] = l2b
    return np.ascontiguousarray(np.tile(row[None, :], (128, 1)))


def kernel(x, c1w, c1b, c2w, c2b, l1w, l1b, l2w, l2b):
    x = np.asarray(x, np.float32)
    wconst = _make_wconst(c1w, c1b, c2w, c2b, l1w, l1b, l2w, l2b)
    nc = _get_nc()
    in_maps = [
        {"x": np.ascontiguousarray(x[i * R : (i + 1) * R]), "wconst": wconst}
        for i in range(N_CORES)
    ]
    res = run_bass_kernel_spmd(nc, in_maps, list(range(N_CORES)))
    return np.concatenate([res.results[i]["out"] for i in range(N_CORES)], axis=0)
